# revision 9
# baseline (speedup 1.0000x reference)
"""Trainium2 Bass kernel for a directed MPNN layer (8 NeuronCores, SPMD).

Reference computation (per edge e = (src, tgt)):
    msg  = relu(edge_hidden @ W_msg.T + b_msg)                     (E, H)
    agg  = segment_sum(msg, tgt, N)                                (N, H)
    excl[e] = sum msg[f] over f with (tgt_f, src_f) == (src_e, tgt_e)
    out[e]  = relu(x[src_e] @ Wx.T + edge_attr[e] @ Wa.T
                   + (agg[src_e] - excl[e]) @ Wm.T + b_upd)
  with W_upd = [Wx | Wa | Wm] split along columns (64 | 16 | 64).

Decomposition (no cross-core communication):
    node_term[v] = x[v] @ Wx.T + agg[v] @ Wm.T + b_upd
    out[e] = relu(node_term[src_e] + edge_attr[e] @ Wa.T - excl[e] @ Wm.T)

  Each core owns 5000 nodes (40 blocks of 128, each split into two
  64-node windows). Edges are reverse pairs (e <-> e +/- E/2), so for
  out-edge e = rev(f), excl[e] = msg[f] (plus rare duplicate-pair
  corrections) and src_e = tgt_f. In-edges sorted by tgt, padded per
  64-node window to khalf chunks of 128; everything for a block runs in
  one fused pass (node_term stays in SBUF):
    msgT for an A/B window-group pair via one block-diagonal matmul,
    msg edge-major + one-hot t4w -> aggT (psum), node_term,
    out[e,h] per chunk = sx-chunk^T Wstack + u2w-win^T nt-win (K=64).
  Duplicate-pair corrections go through a fix-up group whose outputs
  the host splices in (node_term blocks also written to DRAM for its
  indirect gather).  Output is bf16 edge-major (host upcasts).
"""

import numpy as np
import ml_dtypes

import concourse.bacc as bacc
import concourse.bass as bass
import concourse.mybir as mybir
import concourse.tile as tile
from concourse.bass_utils import run_bass_kernel_spmd

F32 = mybir.dt.float32
F32R = mybir.dt.float32r
BF16 = mybir.dt.bfloat16
I32 = mybir.dt.int32
ALU = mybir.AluOpType
ACTF = mybir.ActivationFunctionType
NPBF = ml_dtypes.bfloat16

N = 40000
E = 800000
E2 = E // 2
H = 64
A = 16
NC = 8
P = 128

NPC = N // NC           # 5000 nodes per core
NBLK = 40               # 128-node blocks per core
NPC_PAD = NBLK * P      # 5120
SPEC_CAP = P            # special (correction) rows per core

_CACHE = {}


def _build(khalf: int):
    assert khalf % 4 == 0
    k_blk = 2 * khalf              # chunks per 128-node block
    ngb = k_blk // 4               # groups (of 4 chunks) per block
    ngb2 = ngb // 2                # col-blocks (A/B group pairs)
    wblk = ngb2 * 512              # eh columns per block
    nch = NBLK * k_blk
    l1 = nch * P
    ob = k_blk * H                 # outD columns per block

    nc = bacc.Bacc("TRN2", target_bir_lowering=False, debug=False,
                   num_devices=NC)

    def inp(name, shape, dtype):
        return nc.dram_tensor(name, shape, dtype, kind="ExternalInput").ap()

    eh_g = inp("eh_g", [P, NBLK * wblk], BF16)
    t4w = inp("t4w", [P, nch * H], BF16)       # one-hot [edge, win-node]
    u2w = inp("u2w", [P, nch * H], BF16)       # one-hot [win-node, edge]
    attr_T = inp("attr_T", [A, l1], BF16)      # edge_attr of rev(f), T
    xT_own = inp("xT_own", [H, NPC_PAD], F32R)
    ehF_T = inp("ehF_T", [H, P], BF16)         # correction source rows
    ehRF_T = inp("ehRF_T", [H, P], BF16)       # eh[rev(affected e)], T
    attrF_T = inp("attrF_T", [A, P], BF16)
    Sneg = inp("Sneg", [P, P], F32R)
    didx = inp("didx", [P, 1], I32)
    Wmsg2 = inp("Wmsg2", [P, H], BF16)         # W_msg.T doubled (2x64)
    Wdiag = inp("Wdiag", [P, P], BF16)         # blockdiag(W_msg.T x2)
    Wua = inp("Wua", [A, H], BF16)
    negWum = inp("negWum", [H, H], BF16)
    Wstack = inp("Wstack", [H + A, H], BF16)   # [negWum ; Wua]
    Wum = inp("Wum", [H, H], F32R)
    Wux = inp("Wux", [H, H], F32R)
    bupd = inp("bupd", [1, H], F32R)
    ones1 = inp("ones1", [1, P], F32R)
    ident = inp("ident", [P, P], BF16)

    outD = nc.dram_tensor("outD", [P, NBLK * ob], BF16,
                          kind="ExternalOutput").ap()
    outF = nc.dram_tensor("outF", [H, P], F32, kind="ExternalOutput").ap()
    nt_own = nc.dram_tensor("nt_own", [NPC_PAD, H], BF16).ap()

    with tile.TileContext(nc) as tc:
        with (
            tc.tile_pool(name="const", bufs=1) as cst,
            tc.tile_pool(name="peh", bufs=2) as peh,
            tc.tile_pool(name="psx", bufs=ngb + 2) as psx,
            tc.tile_pool(name="pmsg", bufs=3) as pmsg,
            tc.tile_pool(name="pt4", bufs=3) as pt4,
            tc.tile_pool(name="pu2", bufs=ngb2 + 2) as pu2,
            tc.tile_pool(name="pnt", bufs=2) as pnt,
            tc.tile_pool(name="pout", bufs=2) as pout,
            tc.tile_pool(name="pfix", bufs=1) as pfix,
            tc.tile_pool(name="ps_mT", bufs=2, space="PSUM") as ps_mT,
            tc.tile_pool(name="ps_m4", bufs=1, space="PSUM") as ps_m4,
            tc.tile_pool(name="ps_agg", bufs=1, space="PSUM") as ps_agg,
            tc.tile_pool(name="ps_nt", bufs=1, space="PSUM") as ps_nt,
            tc.tile_pool(name="ps_o", bufs=2, space="PSUM") as ps_o,
        ):
            def load_const(name, ap_in, shape, dtype, eng=None):
                t = cst.tile(shape, dtype, tag=name)
                (eng or nc.scalar).dma_start(t[:], ap_in[:])
                return t

            Wmsg2_sb = load_const("c_wmsg2", Wmsg2, [P, H], BF16)
            Wdiag_sb = load_const("c_wdiag", Wdiag, [P, P], BF16)
            Wua_sb = load_const("c_wua", Wua, [A, H], BF16)
            negWum_sb = load_const("c_nwum", negWum, [H, H], BF16)
            Wstack_sb = load_const("c_wstack", Wstack, [H + A, H], BF16)
            Wum_sb = load_const("c_wum", Wum, [H, H], F32R)
            Wux_sb = load_const("c_wux", Wux, [H, H], F32R)
            bupd_sb = load_const("c_bupd", bupd, [1, H], F32R)
            ones1_sb = load_const("c_ones1", ones1, [1, P], F32R)
            ident_sb = load_const("c_ident", ident, [P, P], BF16)
            xT_sb = load_const("c_xt", xT_own, [H, NPC_PAD], F32R)
            Sneg_sb = load_const("c_sneg", Sneg, [P, P], F32R, nc.gpsimd)
            didx_sb = load_const("c_didx", didx, [P, 1], I32, nc.gpsimd)
            ehF_sb = load_const("c_ehf", ehF_T, [H, P], BF16, nc.gpsimd)
            ehRF_sb = load_const("c_ehrf", ehRF_T, [H, P], BF16, nc.gpsimd)
            attrF_sb = load_const("c_attrf", attrF_T, [A, P], BF16,
                                  nc.gpsimd)

            # b_upd broadcast to 128 partitions via K=1 matmul
            ps_b = ps_nt.tile([P, H], F32, tag="nt")
            nc.tensor.matmul(ps_b[:], lhsT=ones1_sb[:], rhs=bupd_sb[:],
                             start=True, stop=True)
            b_bcast = cst.tile([P, H], F32, tag="c_bb")
            nc.vector.tensor_copy(b_bcast[:], ps_b[:])

            for b in range(NBLK):
                ehb = peh.tile([P, wblk], BF16, tag="eh")
                nc.sync.dma_start(ehb[:], eh_g[:, b * wblk:(b + 1) * wblk])
                agg_A = ps_agg.tile([H, H], F32, tag="aggA")
                agg_B = ps_agg.tile([H, H], F32, tag="aggB")
                agg_w = (agg_A, agg_B)
                sx_tiles = [None] * ngb
                for gp in range(ngb2):
                    cb = gp * 512
                    mT_ps = ps_mT.tile([P, 512], F32, tag="mT")
                    nc.tensor.matmul(mT_ps[:], lhsT=Wdiag_sb[:],
                                     rhs=ehb[:, cb:cb + 512],
                                     start=True, stop=True)
                    for w in range(2):
                        g = gp + w * ngb2
                        half = H * w
                        ch0 = b * k_blk + 4 * g
                        c0 = ch0 * P
                        sx = psx.tile([H + A, 512], BF16, tag="sx")
                        nc.scalar.activation(sx[0:H, :],
                                             mT_ps[half:half + H, :],
                                             ACTF.Relu)
                        nc.gpsimd.dma_start(sx[H:H + A, :],
                                            attr_T[:, c0:c0 + 512])
                        sx_tiles[g] = sx
                        m4_ps = ps_m4.tile([P, 4 * H], F32, tag="m4")
                        for j in range(4):
                            nc.tensor.matmul(
                                m4_ps[:, j * H:(j + 1) * H],
                                lhsT=ehb[half:half + H,
                                         cb + j * P:cb + (j + 1) * P],
                                rhs=Wmsg2_sb[half:half + H, :],
                                start=True, stop=True)
                        m4_sb = pmsg.tile([P, 4 * H], BF16, tag="m4s")
                        nc.vector.tensor_scalar(out=m4_sb[:], in0=m4_ps[:],
                                                scalar1=0.0, scalar2=None,
                                                op0=ALU.max)
                        t4t = pt4.tile([P, 4 * H], BF16, tag="t4")
                        nc.sync.dma_start(t4t[:],
                                          t4w[:, ch0 * H:(ch0 + 4) * H])
                        for j in range(4):
                            nc.tensor.matmul(
                                agg_w[w][:],
                                lhsT=m4_sb[:, j * H:(j + 1) * H],
                                rhs=t4t[:, j * H:(j + 1) * H],
                                start=(gp == 0 and j == 0),
                                stop=(gp == ngb2 - 1 and j == 3))
                # node_term for this block
                aggT_sb = pnt.tile([H, P], F32R, tag="aggT")
                nc.scalar.copy(aggT_sb[:, 0:H], agg_A[:])
                nc.scalar.copy(aggT_sb[:, H:P], agg_B[:])
                nt_ps = ps_nt.tile([P, H], F32, tag="nt")
                nc.tensor.matmul(nt_ps[:], lhsT=aggT_sb[:], rhs=Wum_sb[:],
                                 start=True, stop=False)
                nc.tensor.matmul(nt_ps[:],
                                 lhsT=xT_sb[:, b * P:(b + 1) * P],
                                 rhs=Wux_sb[:],
                                 start=False, stop=True)
                ntb = pnt.tile([P, H], BF16, tag="ntb")
                nc.vector.tensor_tensor(out=ntb[:], in0=nt_ps[:],
                                        in1=b_bcast[:], op=ALU.add)
                nc.sync.dma_start(nt_own[b * P:(b + 1) * P, :], ntb[:])
                # out for this block's edges (edge-major)
                ost = pout.tile([P, ob], BF16, tag="ost")
                u2_tiles = []
                for gp in range(ngb2):
                    u2t = pu2.tile([P, 512], BF16, tag="u2")
                    pc0 = (b * khalf + 4 * gp) * P
                    nc.gpsimd.dma_start(u2t[:], u2w[:, pc0:pc0 + 512])
                    u2_tiles.append(u2t)
                for g in range(ngb):
                    w = g // ngb2
                    half = H * w
                    u2t = u2_tiles[g % ngb2]
                    sx = sx_tiles[g]
                    o_ps = ps_o.tile([P, 4 * H], F32, tag="o")
                    for j in range(4):
                        nc.tensor.matmul(o_ps[:, j * H:(j + 1) * H],
                                         lhsT=sx[:, j * P:(j + 1) * P],
                                         rhs=Wstack_sb[:],
                                         start=True, stop=False)
                        nc.tensor.matmul(o_ps[:, j * H:(j + 1) * H],
                                         lhsT=u2t[half:half + H,
                                                  j * P:(j + 1) * P],
                                         rhs=ntb[half:half + H, :],
                                         start=False, stop=True)
                    nc.vector.tensor_scalar(
                        out=ost[:, g * 4 * H:(g + 1) * 4 * H],
                        in0=o_ps[:], scalar1=0.0, scalar2=None,
                        op0=ALU.max)
                nc.scalar.dma_start(outD[:, b * ob:(b + 1) * ob], ost[:])

            # ---- fix-up for duplicate-pair corrected edges ----
            mF_ps = ps_mT.tile([P, 512], F32, tag="mT")
            nc.tensor.matmul(mF_ps[0:H, 0:P], lhsT=Wmsg2_sb[0:H, :],
                             rhs=ehF_sb[:], start=True, stop=True)
            mFT_sb = pfix.tile([H, P], F32R, tag="mFT")
            nc.vector.tensor_scalar(out=mFT_sb[:], in0=mF_ps[0:H, 0:P],
                                    scalar1=0.0, scalar2=None, op0=ALU.max)
            mV_ps = ps_nt.tile([P, H], F32, tag="nt")
            nc.tensor.matmul(mV_ps[:], lhsT=mFT_sb[:], rhs=Wum_sb[:],
                             start=True, stop=True)
            mV_sb = pfix.tile([P, H], F32R, tag="mV")
            nc.vector.tensor_copy(mV_sb[:], mV_ps[:])
            ntgD_sb = pfix.tile([P, H], BF16, tag="ntg")
            nc.gpsimd.indirect_dma_start(
                out=ntgD_sb[:], out_offset=None, in_=nt_own[:],
                in_offset=bass.IndirectOffsetOnAxis(ap=didx_sb[:, 0:1],
                                                    axis=0),
            )
            ntgD_f = pfix.tile([P, H], F32, tag="ntgf")
            nc.vector.tensor_copy(ntgD_f[:], ntgD_sb[:])
            spec_ps = ps_m4.tile([P, 4 * H], F32, tag="m4")
            nc.tensor.matmul(spec_ps[:, 0:H], lhsT=Sneg_sb[:],
                             rhs=mV_sb[:], start=True, stop=True)
            spec_sb = pfix.tile([P, H], BF16, tag="spec")
            nc.vector.tensor_tensor(out=spec_sb[:], in0=spec_ps[:, 0:H],
                                    in1=ntgD_f[:], op=ALU.add)
            mf_ps = ps_mT.tile([P, 512], F32, tag="mT")
            nc.tensor.matmul(mf_ps[0:H, 0:P], lhsT=Wmsg2_sb[0:H, :],
                             rhs=ehRF_sb[:], start=True, stop=True)
            mfT_sb = pfix.tile([H, P], BF16, tag="mrevT")
            nc.scalar.activation(mfT_sb[:], mf_ps[0:H, 0:P], ACTF.Relu)
            of_ps = ps_o.tile([P, 4 * H], F32, tag="o")
            nc.tensor.matmul(of_ps[0:H, 0:P], lhsT=Wua_sb[:],
                             rhs=attrF_sb[:], start=True, stop=False)
            nc.tensor.matmul(of_ps[0:H, 0:P], lhsT=negWum_sb[:],
                             rhs=mfT_sb[:], start=False, stop=False)
            nc.tensor.matmul(of_ps[0:H, 0:P], lhsT=spec_sb[:],
                             rhs=ident_sb[:], start=False, stop=True)
            outF_sb = pfix.tile([H, P], F32, tag="outF")
            nc.vector.tensor_scalar(out=outF_sb[:], in0=of_ps[0:H, 0:P],
                                    scalar1=0.0, scalar2=None, op0=ALU.max)
            nc.sync.dma_start(outF[:], outF_sb[:])

    nc.compile()
    return nc


def _host_prep(x, edge_attr, edge_hidden, W_msg, b_msg, W_upd, b_upd,
               edge_index):
    src = np.asarray(edge_index[0], dtype=np.int64)
    tgt = np.asarray(edge_index[1], dtype=np.int64)
    eh = np.asarray(edge_hidden, dtype=np.float32)
    ea = np.asarray(edge_attr, dtype=np.float32)
    x = np.asarray(x, dtype=np.float32)
    W_msg = np.asarray(W_msg, dtype=np.float32)
    b_msg = np.asarray(b_msg, dtype=np.float32)
    W_upd = np.asarray(W_upd, dtype=np.float32)
    b_upd = np.asarray(b_upd, dtype=np.float32)
    assert not np.any(b_msg), "nonzero b_msg unsupported by this build"

    # ---- tgt-sort & per-(core, block, window) runs ----
    order = np.argsort(tgt, kind="stable")
    tgt_s = tgt[order]
    nhb = NBLK * 2                       # 64-node windows per core
    bndh = np.empty((NC, nhb, 2), np.int64)
    for c in range(NC):
        for hb in range(nhb):
            lo_n = c * NPC + hb * H
            hi_n = min(c * NPC + (hb + 1) * H, (c + 1) * NPC)
            bndh[c, hb] = (np.searchsorted(tgt_s, lo_n, "left"),
                           np.searchsorted(tgt_s, hi_n, "left"))
    runs = bndh[:, :, 1] - bndh[:, :, 0]
    khalf = int(np.ceil(runs.max() / P))
    khalf = ((khalf + 3) // 4) * 4       # groups of 4 chunks per window
    k_blk = 2 * khalf
    ngb = k_blk // 4
    ngb2 = ngb // 2
    wblk = ngb2 * 512
    nch = NBLK * k_blk
    l1 = nch * P

    # ---- exclusion groups (reference's int logic) ----
    keys = tgt * N + src
    q = src * N + tgt
    order2 = np.argsort(keys, kind="stable")
    sk = keys[order2]
    lo2 = np.searchsorted(sk, q, "left")
    hi2 = np.searchsorted(sk, q, "right")
    eids = np.arange(E, dtype=np.int64)
    rev = np.where(eids < E2, eids + E2, eids - E2)
    simple = (hi2 - lo2 == 1) & (order2[lo2] == rev)
    affected = np.where(~simple)[0]

    Wmsg_io = np.ascontiguousarray(W_msg.T)         # [in, out]
    Wmsg2 = np.concatenate([Wmsg_io, Wmsg_io], axis=0).astype(NPBF)
    Wdiag = np.zeros((P, P), np.float32)
    Wdiag[0:H, 0:H] = Wmsg_io
    Wdiag[H:P, H:P] = Wmsg_io
    Wdiag = Wdiag.astype(NPBF)

    in_maps = []
    meta = []
    for c in range(NC):
        gl = np.zeros(l1, np.int64)      # in-edge f per padded position
        trel = np.full(l1, -1, np.int64)  # block-relative tgt (0..127)
        valid = np.zeros(l1, bool)
        for hb in range(nhb):
            lo, hi = bndh[c, hb]
            n = hi - lo
            base = hb * khalf * P        # window hb occupies khalf chunks
            gl[base:base + n] = order[lo:hi]
            trel[base:base + n] = tgt_s[lo:hi] - (c * NPC + (hb // 2) * P)
            valid[base:base + n] = True

        ehp = eh[gl].astype(NPBF)                     # [l1, 64]
        eh_gc = np.zeros((P, NBLK * wblk), NPBF)
        for b in range(NBLK):
            for g in range(ngb):
                half = H * (g // ngb2)
                cols = b * wblk + (g % ngb2) * 512
                p0 = (b * k_blk + 4 * g) * P
                eh_gc[half:half + H, cols:cols + 512] = ehp[p0:p0 + 512].T

        pos = np.arange(l1)
        lane = pos % P
        ch = pos // P
        blk = ch // k_blk
        chb = ch % k_blk                 # chunk within block
        win = (chb >= khalf).astype(np.int64)
        vrel = np.where(valid, trel - H * win, 0)     # 0..63 within window
        assert vrel[valid].min() >= 0 and vrel[valid].max() < H

        t4 = np.zeros((P, nch * H), np.float32)
        t4[lane[valid], ch[valid] * H + vrel[valid]] = 1.0
        pci = blk * khalf + (chb % khalf)
        u2 = np.zeros((P, nch * H), np.float32)
        u2[H * win[valid] + vrel[valid], pci[valid] * P + lane[valid]] = 1.0

        # out-edge e = rev(f); src_e = tgt_f
        el = rev[gl]
        attr_Tc = np.ascontiguousarray(ea[el].T).astype(NPBF)

        xpad = np.zeros((NPC_PAD, H), np.float32)
        n_x = min(NPC_PAD, N - c * NPC)
        xpad[:n_x] = x[c * NPC:c * NPC + n_x]

        # corrections
        aff_c = affected[(src[affected] >= c * NPC)
                         & (src[affected] < (c + 1) * NPC)]
        f_list, s_cols = [], []
        for d, e in enumerate(aff_c):
            for f in order2[lo2[e]:hi2[e]]:
                if f != rev[e]:
                    f_list.append(f)
                    s_cols.append(d)
        assert len(aff_c) <= SPEC_CAP, len(aff_c)
        assert len(f_list) <= P, len(f_list)
        ehF = np.zeros((P, H), np.float32)
        if f_list:
            ehF[:len(f_list)] = eh[np.asarray(f_list)]
        ehRF = np.zeros((P, H), np.float32)
        attrF = np.zeros((P, A), np.float32)
        if len(aff_c):
            ehRF[:len(aff_c)] = eh[rev[aff_c]]
            attrF[:len(aff_c)] = ea[aff_c]
        Sneg = np.zeros((P, P), np.float32)
        for fi, d in enumerate(s_cols):
            Sneg[fi, d] = -1.0
        didx = np.zeros((P, 1), np.int32)
        didx[:len(aff_c), 0] = src[aff_c] - c * NPC

        in_maps.append({
            "eh_g": eh_gc,
            "t4w": t4.astype(NPBF),
            "u2w": u2.astype(NPBF),
            "attr_T": attr_Tc,
            "xT_own": np.ascontiguousarray(xpad.T),
            "ehF_T": np.ascontiguousarray(ehF.T).astype(NPBF),
            "ehRF_T": np.ascontiguousarray(ehRF.T).astype(NPBF),
            "attrF_T": np.ascontiguousarray(attrF.T).astype(NPBF),
            "Sneg": Sneg,
            "didx": didx,
            "Wmsg2": Wmsg2,
            "Wdiag": Wdiag,
            "Wua": np.ascontiguousarray(W_upd[:, H:H + A].T).astype(NPBF),
            "negWum": np.ascontiguousarray(-W_upd[:, H + A:].T).astype(NPBF),
            "Wstack": np.concatenate(
                [-W_upd[:, H + A:].T, W_upd[:, H:H + A].T],
                axis=0).astype(NPBF),
            "Wum": np.ascontiguousarray(W_upd[:, H + A:].T),
            "Wux": np.ascontiguousarray(W_upd[:, :H].T),
            "bupd": np.ascontiguousarray(b_upd[None, :]),
            "ones1": np.ones((1, P), np.float32),
            "ident": np.eye(P, dtype=np.float32).astype(NPBF),
        })
        meta.append({"el": el, "valid": valid, "aff_c": aff_c})
    return in_maps, meta, khalf


def kernel(**inputs) -> np.ndarray:
    in_maps, meta, khalf = _host_prep(**inputs)
    if khalf not in _CACHE:
        _CACHE[khalf] = _build(khalf)
    nc = _CACHE[khalf]
    res = run_bass_kernel_spmd(nc, in_maps, core_ids=list(range(NC)))
    k_blk = 2 * khalf
    nch = NBLK * k_blk
    l1 = nch * P
    out = np.empty((E, H), np.float32)
    for c in range(NC):
        oD = np.asarray(res.results[c]["outD"], np.float32)
        o = oD.reshape(P, nch, H).transpose(1, 0, 2).reshape(l1, H)
        m = meta[c]
        out[m["el"][m["valid"]]] = o[m["valid"]]
    for c in range(NC):
        oF = res.results[c]["outF"]
        aff_c = meta[c]["aff_c"]
        if len(aff_c):
            out[aff_c] = np.asarray(oF[:, :len(aff_c)].T, np.float32)
    return out


# revision 15
# speedup vs baseline: 1.1336x; 1.1336x over previous
"""Trainium2 Bass kernel for a directed MPNN layer (8 NeuronCores, SPMD).

Reference computation (per edge e = (src, tgt)):
    msg  = relu(edge_hidden @ W_msg.T + b_msg)                     (E, H)
    agg  = segment_sum(msg, tgt, N)                                (N, H)
    excl[e] = sum msg[f] over f with (tgt_f, src_f) == (src_e, tgt_e)
    out[e]  = relu(x[src_e] @ Wx.T + edge_attr[e] @ Wa.T
                   + (agg[src_e] - excl[e]) @ Wm.T + b_upd)
  with W_upd = [Wx | Wa | Wm] split along columns (64 | 16 | 64).

Decomposition (no cross-core communication):
    node_term[v] = x[v] @ Wx.T + agg[v] @ Wm.T + b_upd
    out[e] = relu(node_term[src_e] + edge_attr[e] @ Wa.T - excl[e] @ Wm.T)

  Each core owns 5000 nodes (40 blocks of 128, each split into two
  64-node windows). Edges are reverse pairs (e <-> e +/- E/2), so for
  out-edge e = rev(f), excl[e] = msg[f] (plus rare duplicate-pair
  corrections) and src_e = tgt_f. In-edges sorted by tgt, padded per
  64-node window to khalf chunks of 128; everything for a block runs in
  one fused pass (node_term stays in SBUF):
    msgT for an A/B window-group pair via one block-diagonal matmul,
    msg edge-major + one-hot t4w -> aggT (psum), node_term,
    out[e,h] per chunk = sx-chunk^T Wstack + u2w-win^T nt-win (K=64).
  Duplicate-pair corrections go through a fix-up group whose outputs
  the host splices in (node_term blocks also written to DRAM for its
  indirect gather).  Output is bf16 edge-major (host upcasts).
"""

import numpy as np
import ml_dtypes

import concourse.bacc as bacc
import concourse.bass as bass
import concourse.mybir as mybir
import concourse.tile as tile
from concourse.bass_utils import run_bass_kernel_spmd

F32 = mybir.dt.float32
F32R = mybir.dt.float32r
BF16 = mybir.dt.bfloat16
I32 = mybir.dt.int32
ALU = mybir.AluOpType
ACTF = mybir.ActivationFunctionType
NPBF = ml_dtypes.bfloat16

N = 40000
E = 800000
E2 = E // 2
H = 64
A = 16
NC = 8
P = 128

NPC = N // NC           # 5000 nodes per core
NBLK = 40               # 128-node blocks per core
NPC_PAD = NBLK * P      # 5120
SPEC_CAP = P            # special (correction) rows per core

_CACHE = {}


def _build(khalf: int):
    assert khalf % 4 == 0
    k_blk = 2 * khalf              # chunks per 128-node block
    ngb = k_blk // 4               # groups (of 4 chunks) per block
    ngb2 = ngb // 2                # col-blocks (A/B group pairs)
    wblk = ngb2 * 512              # eh columns per block
    nch = NBLK * k_blk
    l1 = nch * P
    ob = k_blk * H                 # outD columns per block

    nc = bacc.Bacc("TRN2", target_bir_lowering=False, debug=False,
                   num_devices=NC)

    def inp(name, shape, dtype):
        return nc.dram_tensor(name, shape, dtype, kind="ExternalInput").ap()

    eh_g = inp("eh_g", [P, NBLK * wblk], BF16)
    t4w = inp("t4w", [P, nch * H], BF16)       # one-hot [edge, win-node]
    U2 = inp("U2", [P, l1], BF16)              # one-hot [node, edge]
    attr_T = inp("attr_T", [A, l1], BF16)      # edge_attr of rev(f), T
    xT_own = inp("xT_own", [H, NPC_PAD], F32R)
    ehF_T = inp("ehF_T", [H, P], BF16)         # correction source rows
    ehRF_T = inp("ehRF_T", [H, P], BF16)       # eh[rev(affected e)], T
    attrF_T = inp("attrF_T", [A, P], BF16)
    Sneg = inp("Sneg", [P, P], F32R)
    didx = inp("didx", [P, 1], I32)
    Wmsg2 = inp("Wmsg2", [P, H], BF16)         # W_msg.T doubled (2x64)
    Wdiag = inp("Wdiag", [P, P], BF16)         # blockdiag(W_msg.T x2)
    Wua = inp("Wua", [A, H], BF16)
    negWum = inp("negWum", [H, H], BF16)
    Wstack = inp("Wstack", [H + A, H], BF16)   # [negWum ; Wua]
    Wum = inp("Wum", [H, H], F32R)
    Wux = inp("Wux", [H, H], F32R)
    bupd = inp("bupd", [1, H], F32R)
    ones1 = inp("ones1", [1, P], F32R)
    ident = inp("ident", [P, P], BF16)

    outD = nc.dram_tensor("outD", [P, NBLK * ob], BF16,
                          kind="ExternalOutput").ap()
    outF = nc.dram_tensor("outF", [H, P], F32, kind="ExternalOutput").ap()
    nt_own = nc.dram_tensor("nt_own", [NPC_PAD, H], BF16).ap()

    with tile.TileContext(nc) as tc:
        with (
            tc.tile_pool(name="const", bufs=1) as cst,
            tc.tile_pool(name="peh", bufs=2) as peh,
            tc.tile_pool(name="psx", bufs=ngb + 2) as psx,
            tc.tile_pool(name="pmsg", bufs=3) as pmsg,
            tc.tile_pool(name="pt4", bufs=3) as pt4,
            tc.tile_pool(name="pu2", bufs=3) as pu2,
            tc.tile_pool(name="pnt", bufs=2) as pnt,
            tc.tile_pool(name="pout", bufs=2) as pout,
            tc.tile_pool(name="pfix", bufs=1) as pfix,
            tc.tile_pool(name="ps_mT", bufs=2, space="PSUM") as ps_mT,
            tc.tile_pool(name="ps_m4", bufs=1, space="PSUM") as ps_m4,
            tc.tile_pool(name="ps_agg", bufs=1, space="PSUM") as ps_agg,
            tc.tile_pool(name="ps_nt", bufs=1, space="PSUM") as ps_nt,
            tc.tile_pool(name="ps_o", bufs=2, space="PSUM") as ps_o,
        ):
            def load_const(name, ap_in, shape, dtype, eng=None):
                t = cst.tile(shape, dtype, tag=name)
                (eng or nc.scalar).dma_start(t[:], ap_in[:])
                return t

            Wmsg2_sb = load_const("c_wmsg2", Wmsg2, [P, H], BF16)
            Wdiag_sb = load_const("c_wdiag", Wdiag, [P, P], BF16)
            Wua_sb = load_const("c_wua", Wua, [A, H], BF16)
            negWum_sb = load_const("c_nwum", negWum, [H, H], BF16)
            Wstack_sb = load_const("c_wstack", Wstack, [H + A, H], BF16)
            Wum_sb = load_const("c_wum", Wum, [H, H], F32R)
            Wux_sb = load_const("c_wux", Wux, [H, H], F32R)
            bupd_sb = load_const("c_bupd", bupd, [1, H], F32R)
            ones1_sb = load_const("c_ones1", ones1, [1, P], F32R)
            ident_sb = load_const("c_ident", ident, [P, P], BF16)
            xT_sb = load_const("c_xt", xT_own, [H, NPC_PAD], F32R)
            Sneg_sb = load_const("c_sneg", Sneg, [P, P], F32R, nc.gpsimd)
            didx_sb = load_const("c_didx", didx, [P, 1], I32, nc.gpsimd)
            ehF_sb = load_const("c_ehf", ehF_T, [H, P], BF16, nc.gpsimd)
            ehRF_sb = load_const("c_ehrf", ehRF_T, [H, P], BF16, nc.gpsimd)
            attrF_sb = load_const("c_attrf", attrF_T, [A, P], BF16,
                                  nc.gpsimd)

            # b_upd broadcast to 128 partitions via K=1 matmul
            ps_b = ps_nt.tile([P, H], F32, tag="nt")
            nc.tensor.matmul(ps_b[:], lhsT=ones1_sb[:], rhs=bupd_sb[:],
                             start=True, stop=True)
            b_bcast = cst.tile([P, H], F32, tag="c_bb")
            nc.vector.tensor_copy(b_bcast[:], ps_b[:])

            for b in range(NBLK):
                ehb = peh.tile([P, wblk], BF16, tag="eh")
                nc.sync.dma_start(ehb[:], eh_g[:, b * wblk:(b + 1) * wblk])
                agg_A = ps_agg.tile([H, H], F32, tag="aggA")
                agg_B = ps_agg.tile([H, H], F32, tag="aggB")
                agg_w = (agg_A, agg_B)
                sx_tiles = [None] * ngb
                for gp in range(ngb2):
                    cb = gp * 512
                    mT_ps = ps_mT.tile([P, 512], F32, tag="mT")
                    nc.tensor.matmul(mT_ps[:], lhsT=Wdiag_sb[:],
                                     rhs=ehb[:, cb:cb + 512],
                                     start=True, stop=True)
                    for w in range(2):
                        g = gp + w * ngb2
                        half = H * w
                        ch0 = b * k_blk + 4 * g
                        c0 = ch0 * P
                        sx = psx.tile([H + A, 512], BF16, tag="sx")
                        nc.scalar.activation(sx[0:H, :],
                                             mT_ps[half:half + H, :],
                                             ACTF.Relu)
                        nc.gpsimd.dma_start(sx[H:H + A, :],
                                            attr_T[:, c0:c0 + 512])
                        sx_tiles[g] = sx
                        m4_ps = ps_m4.tile([P, 4 * H], F32, tag="m4")
                        for j in range(4):
                            nc.tensor.matmul(
                                m4_ps[:, j * H:(j + 1) * H],
                                lhsT=ehb[half:half + H,
                                         cb + j * P:cb + (j + 1) * P],
                                rhs=Wmsg2_sb[half:half + H, :],
                                start=True, stop=True)
                        m4_sb = pmsg.tile([P, 4 * H], BF16, tag="m4s")
                        nc.vector.tensor_scalar(out=m4_sb[:], in0=m4_ps[:],
                                                scalar1=0.0, scalar2=None,
                                                op0=ALU.max)
                        t4t = pt4.tile([P, 4 * H], BF16, tag="t4")
                        nc.sync.dma_start(t4t[:],
                                          t4w[:, ch0 * H:(ch0 + 4) * H])
                        for j in range(4):
                            nc.tensor.matmul(
                                agg_w[w][:],
                                lhsT=m4_sb[:, j * H:(j + 1) * H],
                                rhs=t4t[:, j * H:(j + 1) * H],
                                start=(gp == 0 and j == 0),
                                stop=(gp == ngb2 - 1 and j == 3))
                # node_term for this block
                aggT_sb = pnt.tile([H, P], F32R, tag="aggT")
                nc.scalar.copy(aggT_sb[:, 0:H], agg_A[:])
                nc.scalar.copy(aggT_sb[:, H:P], agg_B[:])
                nt_ps = ps_nt.tile([P, H], F32, tag="nt")
                nc.tensor.matmul(nt_ps[:], lhsT=aggT_sb[:], rhs=Wum_sb[:],
                                 start=True, stop=False)
                nc.tensor.matmul(nt_ps[:],
                                 lhsT=xT_sb[:, b * P:(b + 1) * P],
                                 rhs=Wux_sb[:],
                                 start=False, stop=True)
                ntb = pnt.tile([P, H], BF16, tag="ntb")
                nc.vector.tensor_tensor(out=ntb[:], in0=nt_ps[:],
                                        in1=b_bcast[:], op=ALU.add)
                nc.sync.dma_start(nt_own[b * P:(b + 1) * P, :], ntb[:])
                # out for this block's edges (feature-major, pair-stacked)
                ost = pout.tile([P, wblk], BF16, tag="ost")
                for g in range(ngb):
                    ch0 = b * k_blk + 4 * g
                    c0 = ch0 * P
                    u2t = pu2.tile([P, 4 * P], BF16, tag="u2")
                    nc.gpsimd.dma_start(u2t[:], U2[:, c0:c0 + 512])
                    sx = sx_tiles[g]
                    o_ps = ps_o.tile([H, 512], F32, tag="o")
                    nc.tensor.matmul(o_ps[:], lhsT=Wstack_sb[:],
                                     rhs=sx[:], start=True, stop=False)
                    nc.tensor.matmul(o_ps[:], lhsT=ntb[:], rhs=u2t[:],
                                     start=False, stop=True)
                    ro = H * (g % 2)
                    co = (g // 2) * 512
                    if g % 2 == 0:
                        nc.vector.tensor_scalar(
                            out=ost[ro:ro + H, co:co + 512], in0=o_ps[:],
                            scalar1=0.0, scalar2=None, op0=ALU.max)
                    else:
                        nc.scalar.activation(ost[ro:ro + H, co:co + 512],
                                             o_ps[:], ACTF.Relu)
                nc.sync.dma_start(outD[:, b * wblk:(b + 1) * wblk], ost[:])

            # ---- fix-up for duplicate-pair corrected edges ----
            mF_ps = ps_mT.tile([P, 512], F32, tag="mT")
            nc.tensor.matmul(mF_ps[0:H, 0:P], lhsT=Wmsg2_sb[0:H, :],
                             rhs=ehF_sb[:], start=True, stop=True)
            mFT_sb = pfix.tile([H, P], F32R, tag="mFT")
            nc.vector.tensor_scalar(out=mFT_sb[:], in0=mF_ps[0:H, 0:P],
                                    scalar1=0.0, scalar2=None, op0=ALU.max)
            mV_ps = ps_nt.tile([P, H], F32, tag="nt")
            nc.tensor.matmul(mV_ps[:], lhsT=mFT_sb[:], rhs=Wum_sb[:],
                             start=True, stop=True)
            mV_sb = pfix.tile([P, H], F32R, tag="mV")
            nc.vector.tensor_copy(mV_sb[:], mV_ps[:])
            ntgD_sb = pfix.tile([P, H], BF16, tag="ntg")
            nc.gpsimd.indirect_dma_start(
                out=ntgD_sb[:], out_offset=None, in_=nt_own[:],
                in_offset=bass.IndirectOffsetOnAxis(ap=didx_sb[:, 0:1],
                                                    axis=0),
            )
            ntgD_f = pfix.tile([P, H], F32, tag="ntgf")
            nc.vector.tensor_copy(ntgD_f[:], ntgD_sb[:])
            spec_ps = ps_m4.tile([P, 4 * H], F32, tag="m4")
            nc.tensor.matmul(spec_ps[:, 0:H], lhsT=Sneg_sb[:],
                             rhs=mV_sb[:], start=True, stop=True)
            spec_sb = pfix.tile([P, H], BF16, tag="spec")
            nc.vector.tensor_tensor(out=spec_sb[:], in0=spec_ps[:, 0:H],
                                    in1=ntgD_f[:], op=ALU.add)
            mf_ps = ps_mT.tile([P, 512], F32, tag="mT")
            nc.tensor.matmul(mf_ps[0:H, 0:P], lhsT=Wmsg2_sb[0:H, :],
                             rhs=ehRF_sb[:], start=True, stop=True)
            mfT_sb = pfix.tile([H, P], BF16, tag="mrevT")
            nc.scalar.activation(mfT_sb[:], mf_ps[0:H, 0:P], ACTF.Relu)
            of_ps = ps_o.tile([P, 4 * H], F32, tag="o")
            nc.tensor.matmul(of_ps[0:H, 0:P], lhsT=Wua_sb[:],
                             rhs=attrF_sb[:], start=True, stop=False)
            nc.tensor.matmul(of_ps[0:H, 0:P], lhsT=negWum_sb[:],
                             rhs=mfT_sb[:], start=False, stop=False)
            nc.tensor.matmul(of_ps[0:H, 0:P], lhsT=spec_sb[:],
                             rhs=ident_sb[:], start=False, stop=True)
            outF_sb = pfix.tile([H, P], F32, tag="outF")
            nc.vector.tensor_scalar(out=outF_sb[:], in0=of_ps[0:H, 0:P],
                                    scalar1=0.0, scalar2=None, op0=ALU.max)
            nc.sync.dma_start(outF[:], outF_sb[:])

    nc.compile()
    return nc


def _host_prep(x, edge_attr, edge_hidden, W_msg, b_msg, W_upd, b_upd,
               edge_index):
    src = np.asarray(edge_index[0], dtype=np.int64)
    tgt = np.asarray(edge_index[1], dtype=np.int64)
    eh = np.asarray(edge_hidden, dtype=np.float32)
    ea = np.asarray(edge_attr, dtype=np.float32)
    x = np.asarray(x, dtype=np.float32)
    W_msg = np.asarray(W_msg, dtype=np.float32)
    b_msg = np.asarray(b_msg, dtype=np.float32)
    W_upd = np.asarray(W_upd, dtype=np.float32)
    b_upd = np.asarray(b_upd, dtype=np.float32)
    assert not np.any(b_msg), "nonzero b_msg unsupported by this build"

    # ---- tgt-sort & per-(core, block, window) runs ----
    order = np.argsort(tgt, kind="stable")
    tgt_s = tgt[order]
    nhb = NBLK * 2                       # 64-node windows per core
    bndh = np.empty((NC, nhb, 2), np.int64)
    for c in range(NC):
        for hb in range(nhb):
            lo_n = c * NPC + hb * H
            hi_n = min(c * NPC + (hb + 1) * H, (c + 1) * NPC)
            bndh[c, hb] = (np.searchsorted(tgt_s, lo_n, "left"),
                           np.searchsorted(tgt_s, hi_n, "left"))
    runs = bndh[:, :, 1] - bndh[:, :, 0]
    khalf = int(np.ceil(runs.max() / P))
    khalf = ((khalf + 3) // 4) * 4       # groups of 4 chunks per window
    k_blk = 2 * khalf
    ngb = k_blk // 4
    ngb2 = ngb // 2
    wblk = ngb2 * 512
    nch = NBLK * k_blk
    l1 = nch * P

    # ---- exclusion groups (reference's int logic) ----
    keys = tgt * N + src
    q = src * N + tgt
    order2 = np.argsort(keys, kind="stable")
    sk = keys[order2]
    lo2 = np.searchsorted(sk, q, "left")
    hi2 = np.searchsorted(sk, q, "right")
    eids = np.arange(E, dtype=np.int64)
    rev = np.where(eids < E2, eids + E2, eids - E2)
    simple = (hi2 - lo2 == 1) & (order2[lo2] == rev)
    affected = np.where(~simple)[0]

    Wmsg_io = np.ascontiguousarray(W_msg.T)         # [in, out]
    Wmsg2 = np.concatenate([Wmsg_io, Wmsg_io], axis=0).astype(NPBF)
    Wdiag = np.zeros((P, P), np.float32)
    Wdiag[0:H, 0:H] = Wmsg_io
    Wdiag[H:P, H:P] = Wmsg_io
    Wdiag = Wdiag.astype(NPBF)

    in_maps = []
    meta = []
    for c in range(NC):
        gl = np.zeros(l1, np.int64)      # in-edge f per padded position
        trel = np.full(l1, -1, np.int64)  # block-relative tgt (0..127)
        valid = np.zeros(l1, bool)
        for hb in range(nhb):
            lo, hi = bndh[c, hb]
            n = hi - lo
            base = hb * khalf * P        # window hb occupies khalf chunks
            gl[base:base + n] = order[lo:hi]
            trel[base:base + n] = tgt_s[lo:hi] - (c * NPC + (hb // 2) * P)
            valid[base:base + n] = True

        ehp = eh[gl].astype(NPBF)                     # [l1, 64]
        eh_gc = np.zeros((P, NBLK * wblk), NPBF)
        for b in range(NBLK):
            for g in range(ngb):
                half = H * (g // ngb2)
                cols = b * wblk + (g % ngb2) * 512
                p0 = (b * k_blk + 4 * g) * P
                eh_gc[half:half + H, cols:cols + 512] = ehp[p0:p0 + 512].T

        pos = np.arange(l1)
        lane = pos % P
        ch = pos // P
        blk = ch // k_blk
        chb = ch % k_blk                 # chunk within block
        win = (chb >= khalf).astype(np.int64)
        vrel = np.where(valid, trel - H * win, 0)     # 0..63 within window
        assert vrel[valid].min() >= 0 and vrel[valid].max() < H

        t4 = np.zeros((P, nch * H), np.float32)
        t4[lane[valid], ch[valid] * H + vrel[valid]] = 1.0
        u2 = np.zeros((P, l1), np.float32)
        u2[trel[valid], pos[valid]] = 1.0

        # out-edge e = rev(f); src_e = tgt_f
        el = rev[gl]
        attr_Tc = np.ascontiguousarray(ea[el].T).astype(NPBF)

        xpad = np.zeros((NPC_PAD, H), np.float32)
        n_x = min(NPC_PAD, N - c * NPC)
        xpad[:n_x] = x[c * NPC:c * NPC + n_x]

        # corrections
        aff_c = affected[(src[affected] >= c * NPC)
                         & (src[affected] < (c + 1) * NPC)]
        f_list, s_cols = [], []
        for d, e in enumerate(aff_c):
            for f in order2[lo2[e]:hi2[e]]:
                if f != rev[e]:
                    f_list.append(f)
                    s_cols.append(d)
        assert len(aff_c) <= SPEC_CAP, len(aff_c)
        assert len(f_list) <= P, len(f_list)
        ehF = np.zeros((P, H), np.float32)
        if f_list:
            ehF[:len(f_list)] = eh[np.asarray(f_list)]
        ehRF = np.zeros((P, H), np.float32)
        attrF = np.zeros((P, A), np.float32)
        if len(aff_c):
            ehRF[:len(aff_c)] = eh[rev[aff_c]]
            attrF[:len(aff_c)] = ea[aff_c]
        Sneg = np.zeros((P, P), np.float32)
        for fi, d in enumerate(s_cols):
            Sneg[fi, d] = -1.0
        didx = np.zeros((P, 1), np.int32)
        didx[:len(aff_c), 0] = src[aff_c] - c * NPC

        in_maps.append({
            "eh_g": eh_gc,
            "t4w": t4.astype(NPBF),
            "U2": u2.astype(NPBF),
            "attr_T": attr_Tc,
            "xT_own": np.ascontiguousarray(xpad.T),
            "ehF_T": np.ascontiguousarray(ehF.T).astype(NPBF),
            "ehRF_T": np.ascontiguousarray(ehRF.T).astype(NPBF),
            "attrF_T": np.ascontiguousarray(attrF.T).astype(NPBF),
            "Sneg": Sneg,
            "didx": didx,
            "Wmsg2": Wmsg2,
            "Wdiag": Wdiag,
            "Wua": np.ascontiguousarray(W_upd[:, H:H + A].T).astype(NPBF),
            "negWum": np.ascontiguousarray(-W_upd[:, H + A:].T).astype(NPBF),
            "Wstack": np.concatenate(
                [-W_upd[:, H + A:].T, W_upd[:, H:H + A].T],
                axis=0).astype(NPBF),
            "Wum": np.ascontiguousarray(W_upd[:, H + A:].T),
            "Wux": np.ascontiguousarray(W_upd[:, :H].T),
            "bupd": np.ascontiguousarray(b_upd[None, :]),
            "ones1": np.ones((1, P), np.float32),
            "ident": np.eye(P, dtype=np.float32).astype(NPBF),
        })
        meta.append({"el": el, "valid": valid, "aff_c": aff_c})
    return in_maps, meta, khalf


def kernel(**inputs) -> np.ndarray:
    in_maps, meta, khalf = _host_prep(**inputs)
    if khalf not in _CACHE:
        _CACHE[khalf] = _build(khalf)
    nc = _CACHE[khalf]
    res = run_bass_kernel_spmd(nc, in_maps, core_ids=list(range(NC)))
    k_blk = 2 * khalf
    nch = NBLK * k_blk
    l1 = nch * P
    ngb = k_blk // 4
    ngb2 = ngb // 2
    wblk = ngb2 * 512
    out = np.empty((E, H), np.float32)
    for c in range(NC):
        oD = np.asarray(res.results[c]["outD"], np.float32)
        oT = np.empty((H, l1), np.float32)
        for b in range(NBLK):
            for g in range(ngb):
                ro = H * (g % 2)
                co = b * wblk + (g // 2) * 512
                p0 = (b * k_blk + 4 * g) * P
                oT[:, p0:p0 + 512] = oD[ro:ro + H, co:co + 512]
        m = meta[c]
        out[m["el"][m["valid"]]] = oT.T[m["valid"]]
    for c in range(NC):
        oF = res.results[c]["outF"]
        aff_c = meta[c]["aff_c"]
        if len(aff_c):
            out[aff_c] = np.asarray(oF[:, :len(aff_c)].T, np.float32)
    return out
